# revision 8
# baseline (speedup 1.0000x reference)
"""Causal self-attention (B=4, T=2048, C=1024, H=16) on 8 Trainium2 cores.

Sharding: core c -> batch b = c//2, head-group g = c%2 (8 heads each,
tensor-parallel). QKV + attention + c_proj computed per core on its head
slice; partial c_proj outputs of a (b) pair are summed with chunked
on-device ReduceScatters over the T dimension; host reassembles.

v2: x is transposed to [C, T] bf16 on the host (no on-device transpose),
and all PE work is emitted as one dense interleaved stream (scores/exp/av
of each unit interleaved with qkv/proj filler) to keep the PE warm.

Self-contained: only imports concourse (installed library) + numpy.
"""

import ml_dtypes
import numpy as np

import concourse.mybir as mybir
import concourse.tile as tile
from concourse import bacc
from concourse.bass_utils import run_bass_kernel_spmd
from concourse.masks import make_identity

B, T, C = 4, 2048, 1024
H_TOTAL, D = 16, 64
N_CORES = 8
HL = H_TOTAL // 2  # local heads per core (8)
HC = HL * D  # local head cols (512)
NP = HL // 2  # head pairs (4)
P = 128
TT = T // P  # 16 t-chunks of 128
CK = C // P  # 8 contraction chunks for qkv
RS_CHUNKS = 8
F32 = mybir.dt.float32
BF16 = mybir.dt.bfloat16
MASK_VAL = -480.0  # -60 after the 1/8 attention scale; exp(-60) ~ 0
SCALE = 1.0 / 8.0  # 1/sqrt(D)

_CACHE = {}


def _build_nc():
    nc = bacc.Bacc("TRN2", target_bir_lowering=False, debug=False, num_devices=N_CORES)

    xT_d = nc.dram_tensor("xT", [P, CK, T], BF16, kind="ExternalInput")
    wq_d = nc.dram_tensor("wq", [P, NP, CK, P], BF16, kind="ExternalInput")
    wk_d = nc.dram_tensor("wk", [P, NP, CK, P], BF16, kind="ExternalInput")
    wv_d = nc.dram_tensor("wv", [P, CK, HC], BF16, kind="ExternalInput")
    bq_d = nc.dram_tensor("bq", [P, NP], F32, kind="ExternalInput")
    bk_d = nc.dram_tensor("bk", [P, NP], F32, kind="ExternalInput")
    bv_d = nc.dram_tensor("bv", [P, HC], F32, kind="ExternalInput")
    wp_d = nc.dram_tensor("wp", [P, HC // P, C], BF16, kind="ExternalInput")
    bp_d = nc.dram_tensor("bp", [P, C], F32, kind="ExternalInput")
    out_d = nc.dram_tensor("out", [T // 2, C], BF16, kind="ExternalOutput")

    with tile.TileContext(nc) as tc:
        with (
            tc.tile_pool(name="const", bufs=1) as constp,
            tc.tile_pool(name="big", bufs=1) as bigp,
            tc.tile_pool(name="xt", bufs=1) as xtp,
            tc.tile_pool(name="prot", bufs=2) as rotp,
            tc.tile_pool(name="wqk", bufs=2) as wqkp,
            tc.tile_pool(name="wpp", bufs=1) as wppp,
            tc.tile_pool(name="wvp", bufs=1) as wvp,
            tc.tile_pool(name="ypair", bufs=2) as ypairp,
            tc.tile_pool(name="work", bufs=4) as workp,
            tc.tile_pool(name="zout", bufs=3) as zoutp,
            tc.tile_pool(name="score_ps", bufs=2, space="PSUM") as score_ps,
            tc.tile_pool(name="small_ps", bufs=2, space="PSUM") as small_ps,
            tc.tile_pool(name="mm_ps", bufs=2, space="PSUM") as mm_ps,
            tc.tile_pool(name="dram", bufs=1, space="DRAM") as dramp,
        ):
            # ---- constants ----
            ident = constp.tile([P, P], F32)
            make_identity(nc, ident)
            ident_bf = constp.tile([P, P], BF16)
            nc.vector.tensor_copy(out=ident_bf[:], in_=ident[:])
            # additive causal mask for the diagonal 128x128 block:
            # mask[s, u] = 0 where u >= s else MASK_VAL
            dmask = constp.tile([P, P], F32)
            nc.gpsimd.memset(dmask, 0.0)
            nc.gpsimd.affine_select(
                out=dmask,
                in_=dmask,
                compare_op=mybir.AluOpType.is_ge,
                fill=MASK_VAL,
                base=0,
                pattern=[[1, P]],
                channel_multiplier=-1,
            )
            bq_sb = constp.tile([P, NP], F32)
            nc.sync.dma_start(bq_sb[:], bq_d[:])
            bk_sb = constp.tile([P, NP], F32)
            nc.sync.dma_start(bk_sb[:], bk_d[:])
            bv_sb = constp.tile([P, HC], F32)
            nc.sync.dma_start(bv_sb[:], bv_d[:])
            bp_sb = constp.tile([P, C], F32)
            nc.sync.dma_start(bp_sb[:], bp_d[:])

            # ---- persistent activations ----
            xT_sb = xtp.tile([P, CK, T], BF16)
            for u4 in range(4):
                nc.sync.dma_start(
                    xT_sb[:, :, u4 * 512 : (u4 + 1) * 512],
                    xT_d[:, :, u4 * 512 : (u4 + 1) * 512],
                )
            qT = bigp.tile([P, NP, T], BF16)  # q^T [qcol, t]
            kT = bigp.tile([P, NP, T], BF16)  # k^T [kcol, t]
            v_ext = bigp.tile([P, TT, HL, D + 1], BF16)  # v with ones col
            yT = bigp.tile([P, NP, T], BF16)  # y^T [ci, t]
            nc.vector.memset(v_ext[:, :, :, D : D + 1], 1.0)

            wv_sb = wvp.tile([P, CK, HC], BF16)
            nc.sync.dma_start(wv_sb[:], wv_d[:])
            wp_sb = wppp.tile([P, HC // P, C], BF16)
            nc.sync.dma_start(wp_sb[:], wp_d[:])
            z_dram = dramp.tile([T, C], BF16)
            rs_out = dramp.tile([T // 2, C], BF16)

            def xT(ck):
                return xT_sb[:, ck]

            # ---- filler task generators (each call emits ~8 matmuls) ----
            def v_task(tt):
                ps = mm_ps.tile([P, 512], F32, tag="mm", name="v_ps")
                for ck in range(CK):
                    nc.tensor.matmul(
                        ps[:],
                        xT(ck)[:, tt * P : (tt + 1) * P],
                        wv_sb[:, ck, :],
                        start=(ck == 0),
                        stop=(ck == CK - 1),
                    )
                nc.vector.tensor_add(
                    out=v_ext[:, tt, :, 0:D],
                    in0=ps[:].rearrange("p (h d) -> p h d", d=D),
                    in1=bv_sb[:].rearrange("p (h d) -> p h d", d=D),
                )

            wqk_tiles = {}

            def qk_dma(j, qk):
                w_d = wq_d if qk == 0 else wk_d
                wj = wqkp.tile(
                    [P, CK, P], BF16, tag=f"w{j}_{qk}", bufs=1, name=f"w{j}_{qk}"
                )
                nc.sync.dma_start(wj[:], w_d[:, j])
                wqk_tiles[(j, qk)] = wj

            def qk_task(j, s):
                # s in 0..7: 0-3 -> q u-chunks 0-3, 4-7 -> k u-chunks 0-3
                qk, u = divmod(s, 4)
                b_sb = bq_sb if qk == 0 else bk_sb
                dstT = qT if qk == 0 else kT
                wj = wqk_tiles[(j, qk)]
                ps = mm_ps.tile([P, 512], F32, tag="mm", name="qk_ps")
                for ck in range(CK):
                    nc.tensor.matmul(
                        ps[:],
                        wj[:, ck, :],
                        xT(ck)[:, u * 512 : (u + 1) * 512],
                        start=(ck == 0),
                        stop=(ck == CK - 1),
                    )
                nc.vector.tensor_add(
                    out=dstT[:, j, u * 512 : (u + 1) * 512],
                    in0=ps[:],
                    in1=b_sb[:, j : j + 1].to_broadcast((P, 512)),
                )

            def pj_task(tt, n):
                ps = mm_ps.tile([P, 512], F32, tag="mm", name="pj_ps")
                for c in range(HC // P):
                    nc.tensor.matmul(
                        ps[:],
                        yT[:, c, tt * P : (tt + 1) * P],
                        wp_sb[:, c, n * 512 : (n + 1) * 512],
                        start=(c == 0),
                        stop=(c == HC // P - 1),
                    )
                z_sb = zoutp.tile([P, 512], BF16, tag="z", name="z_sb")
                nc.vector.tensor_add(
                    out=z_sb[:],
                    in0=ps[:],
                    in1=bp_sb[:, n * 512 : (n + 1) * 512],
                )
                nc.sync.dma_start(
                    z_dram[tt * P : (tt + 1) * P, n * 512 : (n + 1) * 512],
                    z_sb[:],
                )

            rs_rows = T // RS_CHUNKS  # 256
            rs_half = rs_rows // 2  # 128

            def rs_task(rc):
                nc.gpsimd.collective_compute(
                    "ReduceScatter",
                    mybir.AluOpType.add,
                    replica_groups=[[0, 1], [2, 3], [4, 5], [6, 7]],
                    ins=[z_dram[rc * rs_rows : (rc + 1) * rs_rows, :].opt()],
                    outs=[rs_out[rc * rs_half : (rc + 1) * rs_half, :].opt()],
                )
                nc.gpsimd.dma_start(
                    out_d[rc * rs_half : (rc + 1) * rs_half, :],
                    rs_out[rc * rs_half : (rc + 1) * rs_half, :],
                )

            # ---- attention unit pieces ----
            def sc_group(j, u, i, p_a, p_b):
                # score matmuls + exp for key-chunk i of unit (j, u)
                ps2 = [
                    score_ps.tile([P, 1024], F32, tag="score", name=f"sc{hh}")
                    for hh in range(2)
                ]
                for jj in range(2 * u, 2 * u + 2):
                    if jj < i // 4:
                        continue
                    c0 = (jj - 2 * u) * 512
                    for hh in range(2):  # adjacent emission -> row-group packing
                        hb = hh * D
                        nc.tensor.matmul(
                            ps2[hh][:, c0 : c0 + 512],
                            kT[hb : hb + D, j, i * P : (i + 1) * P],
                            qT[hb : hb + D, j, jj * 512 : (jj + 1) * 512],
                            start=True,
                            stop=True,
                        )
                for hh, p_sb in ((0, p_a), (1, p_b)):
                    if i // 8 == u:
                        d0 = i * P - 1024 * u
                        nc.vector.tensor_add(
                            out=ps2[hh][:, d0 : d0 + P],
                            in0=ps2[hh][:, d0 : d0 + P],
                            in1=dmask[:],
                        )
                    c0 = max(0, i * P - 1024 * u)
                    nc.scalar.activation(
                        out=p_sb[:, i, c0:1024],
                        in_=ps2[hh][:, c0:1024],
                        func=mybir.ActivationFunctionType.Exp,
                        scale=SCALE,
                    )

            def av_block(j, u, tl, p_a, p_b):
                # y for global t-chunk 8u+tl, heads (2j, 2j+1)
                t_chunk = 8 * u + tl
                y_pair = ypairp.tile([P, P], BF16, tag="yp", name="y_pair")
                for hh, p_sb in ((0, p_a), (1, p_b)):
                    h = 2 * j + hh
                    ps_av = small_ps.tile([P, D + 1], F32, tag="sp", name="av_ps")
                    for i in range(t_chunk + 1):
                        nc.tensor.matmul(
                            ps_av[:],
                            p_sb[:, i, tl * P : (tl + 1) * P],
                            v_ext[:, i, h, :],
                            start=(i == 0),
                            stop=(i == t_chunk),
                        )
                    recip = workp.tile([P, 1], F32, tag="recip", name="recip")
                    nc.vector.reciprocal(recip[:], ps_av[:, D : D + 1])
                    nc.vector.tensor_mul(
                        out=y_pair[:, hh * D : (hh + 1) * D],
                        in0=ps_av[:, 0:D],
                        in1=recip[:, 0:1].to_broadcast((P, D)),
                    )
                ps = small_ps.tile([P, P], BF16, tag="sp", name="yt_ps")
                nc.tensor.transpose(ps[:], y_pair[:], ident_bf[:])
                nc.vector.tensor_copy(
                    out=yT[:, j, t_chunk * P : (t_chunk + 1) * P],
                    in_=ps[:],
                )

            # ---- schedule ----
            # One dense PE stream. Units run u-major; each unit's av blocks
            # run inside the unit (one key-chunk behind the exp stream).
            # qkproj is split in half (q/k u-chunks 0-1 before the u=0
            # units that need them; u-chunks 2-3 deferred into later units
            # to balance PE load against the ACT-bound exp stream).
            def unit(j, u, fillers, after_av=None):
                n_i = 8 * (u + 1)
                p_a = rotp.tile([P, TT, 1024], BF16, tag="p", name=f"pa{j}{u}")
                p_b = rotp.tile([P, TT, 1024], BF16, tag="p", name=f"pb{j}{u}")
                fi = 0
                for i in range(n_i):
                    sc_group(j, u, i, p_a, p_b)
                    # av block (one behind the exp stream)
                    tl = i - 1 - 8 * u
                    if 0 <= tl < 8:
                        av_block(j, u, tl, p_a, p_b)
                        if after_av is not None:
                            after_av(tl)
                    # spread fillers across the unit
                    want = (i + 1) * len(fillers) // n_i
                    while fi < want:
                        fillers[fi]()
                        fi += 1
                av_block(j, u, 7, p_a, p_b)
                if after_av is not None:
                    after_av(7)

            mk = lambda f, *a: (lambda: f(*a))

            # prelude: q/k weights for pair 0, its first-half proj, v chunk 0
            qk_dma(0, 0)
            qk_dma(0, 1)
            for s in (0, 1, 4, 5):
                qk_task(0, s)
            v_task(0)

            unit(0, 0,
                 [mk(v_task, t) for t in range(1, 8)]
                 + [mk(qk_dma, 1, 0), mk(qk_dma, 1, 1)]
                 + [mk(qk_task, 1, s) for s in (0, 1, 4, 5)])
            unit(1, 0,
                 [mk(v_task, 8), mk(v_task, 9)]
                 + [mk(qk_dma, 2, 0), mk(qk_dma, 2, 1)]
                 + [mk(qk_task, 2, s) for s in (0, 1, 4, 5)]
                 + [mk(qk_task, 0, 2), mk(qk_task, 0, 6)])
            unit(2, 0,
                 [mk(v_task, 10), mk(v_task, 11)]
                 + [mk(qk_dma, 3, 0), mk(qk_dma, 3, 1)]
                 + [mk(qk_task, 3, s) for s in (0, 1, 4, 5)]
                 + [mk(qk_task, 0, 3), mk(qk_task, 0, 7)])
            unit(3, 0,
                 [mk(v_task, t) for t in range(12, 16)]
                 + [mk(qk_task, 1, s) for s in (2, 6, 3, 7)])
            # u=1 region: proj chunks for u=0 t-rows are ready now.
            unit(0, 1,
                 [mk(qk_task, 2, s) for s in (2, 6, 3, 7)]
                 + [mk(pj_task, 0, 0), mk(pj_task, 0, 1),
                    mk(pj_task, 1, 0), mk(pj_task, 1, 1), mk(rs_task, 0),
                    mk(pj_task, 2, 0), mk(pj_task, 2, 1),
                    mk(pj_task, 3, 0), mk(pj_task, 3, 1), mk(rs_task, 1)])
            unit(1, 1,
                 [mk(qk_task, 3, s) for s in (2, 6, 3, 7)]
                 + [mk(pj_task, 4, 0), mk(pj_task, 4, 1),
                    mk(pj_task, 5, 0), mk(pj_task, 5, 1), mk(rs_task, 2)])
            unit(2, 1,
                 [mk(pj_task, 6, 0), mk(pj_task, 6, 1),
                    mk(pj_task, 7, 0), mk(pj_task, 7, 1), mk(rs_task, 3)])

            # last unit: u=1 proj rows become ready as its av completes
            def tail_after_av(tl):
                tt = 8 + tl
                if tl <= 5:
                    pj_task(tt, 0)
                    pj_task(tt, 1)
                if tl in (1, 3, 5):
                    rs_task(4 + (tl - 1) // 2)

            unit(3, 1, [], after_av=tail_after_av)
            for tt in (14, 15):
                pj_task(tt, 0)
                pj_task(tt, 1)
            rs_task(7)

    nc.compile()
    return nc


def _in_maps(inputs):
    x = np.ascontiguousarray(inputs["x"], dtype=np.float32)
    w_attn = np.asarray(inputs["w_attn"], dtype=np.float32)
    b_attn = np.asarray(inputs["b_attn"], dtype=np.float32)
    w_proj = np.asarray(inputs["w_proj"], dtype=np.float32)
    b_proj = np.asarray(inputs["b_proj"], dtype=np.float32)

    # xT per batch: [C, T] -> [ki, ck, t]
    xT_b = [
        np.ascontiguousarray(
            x[b].T.reshape(CK, P, T).transpose(1, 0, 2).astype(ml_dtypes.bfloat16)
        )
        for b in range(B)
    ]

    maps = []
    for core in range(N_CORES):
        b, g = core // 2, core % 2
        s = g * HC
        # [C, HC] -> [ki, j, ko, n] with c = ko*128+ki, qcol = j*128+n
        wq = (
            w_attn[:, s : s + HC]
            .reshape(CK, P, NP, P)
            .transpose(1, 2, 0, 3)
            .astype(ml_dtypes.bfloat16)
        )
        wk = (
            w_attn[:, C + s : C + s + HC]
            .reshape(CK, P, NP, P)
            .transpose(1, 2, 0, 3)
            .astype(ml_dtypes.bfloat16)
        )
        # [C, HC] -> [ki, ko, vcol]
        wv = (
            w_attn[:, 2 * C + s : 2 * C + s + HC]
            .reshape(CK, P, HC)
            .transpose(1, 0, 2)
            .astype(ml_dtypes.bfloat16)
        )
        # [HC, C] -> [ki, ko, co], bf16
        wp = (
            w_proj[s : s + HC, :]
            .reshape(HC // P, P, C)
            .transpose(1, 0, 2)
            .astype(ml_dtypes.bfloat16)
        )
        bq = b_attn[s : s + HC].reshape(NP, P).T
        bk = b_attn[C + s : C + s + HC].reshape(NP, P).T
        bv = np.broadcast_to(b_attn[2 * C + s : 2 * C + s + HC], (P, HC))
        bp = (
            np.broadcast_to(b_proj, (P, C))
            if g == 0
            else np.zeros((P, C), np.float32)
        )
        maps.append(
            {
                "xT": xT_b[b],
                "wq": np.ascontiguousarray(wq),
                "wk": np.ascontiguousarray(wk),
                "wv": np.ascontiguousarray(wv),
                "wp": np.ascontiguousarray(wp),
                "bq": np.ascontiguousarray(bq),
                "bk": np.ascontiguousarray(bk),
                "bv": np.ascontiguousarray(bv),
                "bp": np.ascontiguousarray(bp),
            }
        )
    return maps


def _run(inputs, trace=False, trace_cores=None):
    if "nc" not in _CACHE:
        _CACHE["nc"] = _build_nc()
    nc = _CACHE["nc"]
    res = run_bass_kernel_spmd(
        nc,
        _in_maps(inputs),
        list(range(N_CORES)),
        trace=trace,
        trace_cores=trace_cores,
    )
    # chunked RS ownership: even core holds rows [256c, 256c+128),
    # odd core holds rows [256c+128, 256c+256), for c = 0..7
    out = np.empty((B, T, C), np.float32)
    rows = T // RS_CHUNKS
    half = rows // 2
    for b in range(B):
        ev = res.results[2 * b]["out"].astype(np.float32)
        od = res.results[2 * b + 1]["out"].astype(np.float32)
        for rc in range(RS_CHUNKS):
            out[b, rc * rows : rc * rows + half] = ev[rc * half : (rc + 1) * half]
            out[b, rc * rows + half : (rc + 1) * rows] = od[
                rc * half : (rc + 1) * half
            ]
    return out, res


def kernel(**inputs):
    out, _ = _run(inputs)
    return out


# revision 14
# speedup vs baseline: 1.0270x; 1.0270x over previous
"""Causal self-attention (B=4, T=2048, C=1024, H=16) on 8 Trainium2 cores.

Sharding: core c -> batch b = c//2, head-group g = c%2 (8 heads each,
tensor-parallel). QKV + attention + c_proj computed per core on its head
slice; partial c_proj outputs of a (b) pair are summed with chunked
on-device ReduceScatters over the T dimension; host reassembles.

v2: x is transposed to [C, T] bf16 on the host (no on-device transpose),
and all PE work is emitted as one dense interleaved stream (scores/exp/av
of each unit interleaved with qkv/proj filler) to keep the PE warm.

Self-contained: only imports concourse (installed library) + numpy.
"""

import ml_dtypes
import numpy as np

import concourse.mybir as mybir
import concourse.tile as tile
from concourse import bacc
from concourse.bass_utils import run_bass_kernel_spmd
from concourse.masks import make_identity

B, T, C = 4, 2048, 1024
H_TOTAL, D = 16, 64
N_CORES = 8
HL = H_TOTAL // 2  # local heads per core (8)
HC = HL * D  # local head cols (512)
NP = HL // 2  # head pairs (4)
P = 128
TT = T // P  # 16 t-chunks of 128
CK = C // P  # 8 contraction chunks for qkv
# ReduceScatter chunks as (t_chunk_start, n_t_chunks): big early, small late
RS_LAYOUT = [(0, 4), (4, 4), (8, 4), (12, 2), (14, 1), (15, 1)]
F32 = mybir.dt.float32
BF16 = mybir.dt.bfloat16
MASK_VAL = -480.0  # -60 after the 1/8 attention scale; exp(-60) ~ 0
SCALE = 1.0 / 8.0  # 1/sqrt(D)

_CACHE = {}


def _build_nc():
    nc = bacc.Bacc("TRN2", target_bir_lowering=False, debug=False, num_devices=N_CORES)

    xT_d = nc.dram_tensor("xT", [P, CK, T], BF16, kind="ExternalInput")
    wq_d = nc.dram_tensor("wq", [P, NP, CK, P], BF16, kind="ExternalInput")
    wk_d = nc.dram_tensor("wk", [P, NP, CK, P], BF16, kind="ExternalInput")
    wv_d = nc.dram_tensor("wv", [P, CK, HC], BF16, kind="ExternalInput")
    bq_d = nc.dram_tensor("bq", [P, NP], F32, kind="ExternalInput")
    bk_d = nc.dram_tensor("bk", [P, NP], F32, kind="ExternalInput")
    bv_d = nc.dram_tensor("bv", [P, HC], F32, kind="ExternalInput")
    wp_d = nc.dram_tensor("wp", [P, HC // P, C], BF16, kind="ExternalInput")
    bp_d = nc.dram_tensor("bp", [P, C], F32, kind="ExternalInput")
    out_d = nc.dram_tensor("out", [T // 2, C], BF16, kind="ExternalOutput")

    with tile.TileContext(nc) as tc:
        with (
            tc.tile_pool(name="const", bufs=1) as constp,
            tc.tile_pool(name="big", bufs=1) as bigp,
            tc.tile_pool(name="xt", bufs=1) as xtp,
            tc.tile_pool(name="prot", bufs=2) as rotp,
            tc.tile_pool(name="wqk", bufs=2) as wqkp,
            tc.tile_pool(name="wpp", bufs=1) as wppp,
            tc.tile_pool(name="wvp", bufs=1) as wvp,
            tc.tile_pool(name="ypair", bufs=2) as ypairp,
            tc.tile_pool(name="work", bufs=4) as workp,
            tc.tile_pool(name="zout", bufs=4) as zoutp,
            tc.tile_pool(name="score_ps", bufs=2, space="PSUM") as score_ps,
            tc.tile_pool(name="small_ps", bufs=2, space="PSUM") as small_ps,
            tc.tile_pool(name="mm_ps", bufs=2, space="PSUM") as mm_ps,
            tc.tile_pool(name="dram", bufs=1, space="DRAM") as dramp,
        ):
            # ---- DMAs first: the Sync queue feeds the first matmuls ----
            wq00 = wqkp.tile([P, CK, P], BF16, tag="w0_0", bufs=1, name="w0_0")
            nc.sync.dma_start(wq00[:], wq_d[:, 0])
            xT_sb = xtp.tile([P, CK, T], BF16)
            nc.sync.dma_start(xT_sb[:, :, 0:512], xT_d[:, :, 0:512])
            wv_sb = wvp.tile([P, CK, HC], BF16)
            nc.sync.dma_start(wv_sb[:], wv_d[:])
            wk00 = wqkp.tile([P, CK, P], BF16, tag="w0_1", bufs=1, name="w0_1")
            nc.sync.dma_start(wk00[:], wk_d[:, 0])
            bq_sb = constp.tile([P, NP], F32)
            nc.sync.dma_start(bq_sb[:], bq_d[:])
            bk_sb = constp.tile([P, NP], F32)
            nc.sync.dma_start(bk_sb[:], bk_d[:])
            bv_sb = constp.tile([P, HC], F32)
            nc.sync.dma_start(bv_sb[:], bv_d[:])
            for u4 in range(1, 4):
                nc.sync.dma_start(
                    xT_sb[:, :, u4 * 512 : (u4 + 1) * 512],
                    xT_d[:, :, u4 * 512 : (u4 + 1) * 512],
                )
            bp_sb = constp.tile([P, C], F32)
            nc.sync.dma_start(bp_sb[:], bp_d[:])
            wp_sb = wppp.tile([P, HC // P, C], BF16)
            nc.sync.dma_start(wp_sb[:], wp_d[:])

            # ---- constants ----
            ident = constp.tile([P, P], F32)
            make_identity(nc, ident)
            ident_bf = constp.tile([P, P], BF16)
            nc.vector.tensor_copy(out=ident_bf[:], in_=ident[:])
            # additive causal mask for the diagonal 128x128 block:
            # mask[s, u] = 0 where u >= s else MASK_VAL
            dmask = constp.tile([P, P], F32)
            nc.gpsimd.memset(dmask, 0.0)
            nc.gpsimd.affine_select(
                out=dmask,
                in_=dmask,
                compare_op=mybir.AluOpType.is_ge,
                fill=MASK_VAL,
                base=0,
                pattern=[[1, P]],
                channel_multiplier=-1,
            )

            # ---- persistent activations ----
            qT = bigp.tile([P, NP, T], BF16)  # q^T [qcol, t]
            kT = bigp.tile([P, NP, T], BF16)  # k^T [kcol, t]
            v_ext = bigp.tile([P, TT, HL, D + 1], BF16)  # v with ones col
            yT = bigp.tile([P, NP, T], BF16)  # y^T [ci, t]
            nc.vector.memset(v_ext[:, :, :, D : D + 1], 1.0)

            z_dram = dramp.tile([T, C], BF16)
            rs_out = dramp.tile([T // 2, C], BF16)

            def xT(ck):
                return xT_sb[:, ck]

            # ---- filler task generators (each call emits ~8 matmuls) ----
            def v_task(tt):
                ps = mm_ps.tile([P, 512], F32, tag="mm", name="v_ps")
                for ck in range(CK):
                    nc.tensor.matmul(
                        ps[:],
                        xT(ck)[:, tt * P : (tt + 1) * P],
                        wv_sb[:, ck, :],
                        start=(ck == 0),
                        stop=(ck == CK - 1),
                    )
                nc.vector.tensor_add(
                    out=v_ext[:, tt, :, 0:D],
                    in0=ps[:].rearrange("p (h d) -> p h d", d=D),
                    in1=bv_sb[:].rearrange("p (h d) -> p h d", d=D),
                )

            wqk_tiles = {(0, 0): wq00, (0, 1): wk00}

            def qk_dma(j, qk):
                w_d = wq_d if qk == 0 else wk_d
                wj = wqkp.tile(
                    [P, CK, P], BF16, tag=f"w{j}_{qk}", bufs=1, name=f"w{j}_{qk}"
                )
                nc.sync.dma_start(wj[:], w_d[:, j])
                wqk_tiles[(j, qk)] = wj

            def qk_task(j, s):
                # s in 0..7: 0-3 -> q u-chunks 0-3, 4-7 -> k u-chunks 0-3
                qk, u = divmod(s, 4)
                b_sb = bq_sb if qk == 0 else bk_sb
                dstT = qT if qk == 0 else kT
                wj = wqk_tiles[(j, qk)]
                ps = mm_ps.tile([P, 512], F32, tag="mm", name="qk_ps")
                for ck in range(CK):
                    nc.tensor.matmul(
                        ps[:],
                        wj[:, ck, :],
                        xT(ck)[:, u * 512 : (u + 1) * 512],
                        start=(ck == 0),
                        stop=(ck == CK - 1),
                    )
                nc.vector.tensor_add(
                    out=dstT[:, j, u * 512 : (u + 1) * 512],
                    in0=ps[:],
                    in1=b_sb[:, j : j + 1].to_broadcast((P, 512)),
                )

            def pj_task(tt, n):
                ps = mm_ps.tile([P, 512], F32, tag="mm", name="pj_ps")
                for c in range(HC // P):
                    nc.tensor.matmul(
                        ps[:],
                        yT[:, c, tt * P : (tt + 1) * P],
                        wp_sb[:, c, n * 512 : (n + 1) * 512],
                        start=(c == 0),
                        stop=(c == HC // P - 1),
                    )
                z_sb = zoutp.tile([P, 512], BF16, tag="z", name="z_sb")
                nc.vector.tensor_add(
                    out=z_sb[:],
                    in0=ps[:],
                    in1=bp_sb[:, n * 512 : (n + 1) * 512],
                )
                nc.sync.dma_start(
                    z_dram[tt * P : (tt + 1) * P, n * 512 : (n + 1) * 512],
                    z_sb[:],
                )

            def rs_task(rc):
                # uneven chunks: big ones early (lots of overlap slack),
                # small ones late (they gate the kernel tail)
                t0r, tn = RS_LAYOUT[rc]
                r0, rn = t0r * P, tn * P
                nc.gpsimd.collective_compute(
                    "ReduceScatter",
                    mybir.AluOpType.add,
                    replica_groups=[[0, 1], [2, 3], [4, 5], [6, 7]],
                    ins=[z_dram[r0 : r0 + rn, :].opt()],
                    outs=[rs_out[r0 // 2 : (r0 + rn) // 2, :].opt()],
                )
                nc.gpsimd.dma_start(
                    out_d[r0 // 2 : (r0 + rn) // 2, :],
                    rs_out[r0 // 2 : (r0 + rn) // 2, :],
                )

            # ---- attention unit pieces ----
            def sc_group(j, u, i, p_a, p_b):
                # score matmuls + exp for key-chunk i of unit (j, u)
                ps2 = [
                    score_ps.tile([P, 1024], F32, tag="score", name=f"sc{hh}")
                    for hh in range(2)
                ]
                for jj in range(2 * u, 2 * u + 2):
                    if jj < i // 4:
                        continue
                    c0 = (jj - 2 * u) * 512
                    for hh in range(2):  # adjacent emission -> row-group packing
                        hb = hh * D
                        nc.tensor.matmul(
                            ps2[hh][:, c0 : c0 + 512],
                            kT[hb : hb + D, j, i * P : (i + 1) * P],
                            qT[hb : hb + D, j, jj * 512 : (jj + 1) * 512],
                            start=True,
                            stop=True,
                        )
                for hh, p_sb in ((0, p_a), (1, p_b)):
                    if i // 8 == u:
                        d0 = i * P - 1024 * u
                        nc.vector.tensor_add(
                            out=ps2[hh][:, d0 : d0 + P],
                            in0=ps2[hh][:, d0 : d0 + P],
                            in1=dmask[:],
                        )
                    c0 = max(0, i * P - 1024 * u)
                    nc.scalar.activation(
                        out=p_sb[:, i, c0:1024],
                        in_=ps2[hh][:, c0:1024],
                        func=mybir.ActivationFunctionType.Exp,
                        scale=SCALE,
                    )

            def av_block(j, u, tl, p_a, p_b):
                # y for global t-chunk 8u+tl, heads (2j, 2j+1)
                t_chunk = 8 * u + tl
                y_pair = ypairp.tile([P, P], BF16, tag="yp", name="y_pair")
                for hh, p_sb in ((0, p_a), (1, p_b)):
                    h = 2 * j + hh
                    ps_av = small_ps.tile([P, D + 1], F32, tag="sp", name="av_ps")
                    for i in range(t_chunk + 1):
                        nc.tensor.matmul(
                            ps_av[:],
                            p_sb[:, i, tl * P : (tl + 1) * P],
                            v_ext[:, i, h, :],
                            start=(i == 0),
                            stop=(i == t_chunk),
                        )
                    recip = workp.tile([P, 1], F32, tag="recip", name="recip")
                    nc.vector.reciprocal(recip[:], ps_av[:, D : D + 1])
                    nc.vector.tensor_mul(
                        out=y_pair[:, hh * D : (hh + 1) * D],
                        in0=ps_av[:, 0:D],
                        in1=recip[:, 0:1].to_broadcast((P, D)),
                    )
                ps = small_ps.tile([P, P], BF16, tag="sp", name="yt_ps")
                nc.tensor.transpose(ps[:], y_pair[:], ident_bf[:])
                nc.vector.tensor_copy(
                    out=yT[:, j, t_chunk * P : (t_chunk + 1) * P],
                    in_=ps[:],
                )

            # ---- schedule ----
            # One dense PE stream. Units run u-major; each unit's av blocks
            # run inside the unit (one key-chunk behind the exp stream).
            # Fillers (v/qkproj/proj work) are paced by the unit's exp-time
            # budget so the scalar engine never starves in the exp-bound
            # region; fillers with an intra-unit deadline (v chunks feeding
            # this unit's av) are emitted by their deadline regardless.
            def unit(j, u, fillers, after_av=None):
                n_i = 8 * (u + 1)
                p_a = rotp.tile([P, TT, 1024], BF16, tag="p", name=f"pa{j}{u}")
                p_b = rotp.tile([P, TT, 1024], BF16, tag="p", name=f"pb{j}{u}")
                fi = 0
                budget = spent = 0.0
                for i in range(n_i):
                    c0 = max(0, i * P - 1024 * u)
                    n_mm = sum(2 for jj in range(2 * u, 2 * u + 2) if jj >= i // 4)
                    sc_group(j, u, i, p_a, p_b)
                    spent += n_mm * 0.22
                    budget += 2 * (1024 - c0 + 352) / 1200.0
                    tl = i - 1 - 8 * u
                    if 0 <= tl < 8:
                        av_block(j, u, tl, p_a, p_b)
                        spent += (8 * u + tl + 1) * 0.1 + 0.4
                        if after_av is not None:
                            after_av(tl)
                    while fi < len(fillers) and (
                        (fillers[fi][2] is not None and fillers[fi][2] <= i)
                        or spent + fillers[fi][1] <= budget
                    ):
                        fillers[fi][0]()
                        spent += fillers[fi][1]
                        fi += 1
                av_block(j, u, 7, p_a, p_b)
                if after_av is not None:
                    after_av(7)
                while fi < len(fillers):
                    fillers[fi][0]()
                    fi += 1

            def mk(f, *a):
                return lambda: f(*a)

            V, QK, DMA, PJ, RS = 1.75, 1.75, 0.1, 0.95, 0.1

            # prelude: pair-0 q/k (u-chunks 0,1) + v chunk 0
            for s in (0, 1, 4, 5):
                qk_task(0, s)
            v_task(0)

            unit(0, 0,
                 [(mk(v_task, t), V, t) for t in range(1, 8)]
                 + [(mk(qk_dma, 1, 0), DMA, None), (mk(qk_dma, 1, 1), DMA, None)]
                 + [(mk(qk_task, 1, s), QK, None) for s in (0, 1, 4, 5)])
            unit(1, 0,
                 [(mk(v_task, 8), V, None), (mk(v_task, 9), V, None),
                  (mk(qk_dma, 2, 0), DMA, None), (mk(qk_dma, 2, 1), DMA, None)]
                 + [(mk(qk_task, 2, s), QK, None) for s in (0, 1, 4, 5)]
                 + [(mk(qk_task, 0, 2), QK, None), (mk(qk_task, 0, 6), QK, None)])
            unit(2, 0,
                 [(mk(v_task, 10), V, None), (mk(v_task, 11), V, None),
                  (mk(qk_dma, 3, 0), DMA, None), (mk(qk_dma, 3, 1), DMA, None)]
                 + [(mk(qk_task, 3, s), QK, None) for s in (0, 1, 4, 5)]
                 + [(mk(qk_task, 0, 3), QK, None), (mk(qk_task, 0, 7), QK, None)])
            unit(3, 0,
                 [(mk(v_task, t), V, None) for t in range(12, 16)]
                 + [(mk(qk_task, 1, s), QK, None) for s in (2, 6, 3, 7)])
            # u=1 region: proj chunks for u=0 t-rows are ready now.
            unit(0, 1,
                 [(mk(qk_task, 2, s), QK, None) for s in (2, 6, 3, 7)]
                 + [(mk(pj_task, tt, n), PJ, None)
                    for tt in (0, 1, 2, 3) for n in range(2)]
                 + [(mk(rs_task, 0), RS, None)])
            unit(1, 1,
                 [(mk(qk_task, 3, s), QK, None) for s in (2, 6, 3, 7)]
                 + [(mk(pj_task, tt, n), PJ, None)
                    for tt in (4, 5) for n in range(2)])
            unit(2, 1,
                 [(mk(pj_task, tt, n), PJ, None)
                  for tt in (6, 7) for n in range(2)]
                 + [(mk(rs_task, 1), RS, None)])

            # last unit: u=1 proj rows become ready as its av completes
            def tail_after_av(tl):
                tt = 8 + tl
                if tl <= 6:
                    pj_task(tt, 0)
                    pj_task(tt, 1)
                if tl == 3:
                    rs_task(2)
                elif tl == 5:
                    rs_task(3)
                elif tl == 6:
                    rs_task(4)
                elif tl == 7:
                    pj_task(15, 0)
                    pj_task(15, 1)
                    rs_task(5)

            unit(3, 1, [], after_av=tail_after_av)

    nc.compile()
    return nc


def _in_maps(inputs):
    x = np.ascontiguousarray(inputs["x"], dtype=np.float32)
    w_attn = np.asarray(inputs["w_attn"], dtype=np.float32)
    b_attn = np.asarray(inputs["b_attn"], dtype=np.float32)
    w_proj = np.asarray(inputs["w_proj"], dtype=np.float32)
    b_proj = np.asarray(inputs["b_proj"], dtype=np.float32)

    # xT per batch: [C, T] -> [ki, ck, t]
    xT_b = [
        np.ascontiguousarray(
            x[b].T.reshape(CK, P, T).transpose(1, 0, 2).astype(ml_dtypes.bfloat16)
        )
        for b in range(B)
    ]

    maps = []
    for core in range(N_CORES):
        b, g = core // 2, core % 2
        s = g * HC
        # [C, HC] -> [ki, j, ko, n] with c = ko*128+ki, qcol = j*128+n
        wq = (
            w_attn[:, s : s + HC]
            .reshape(CK, P, NP, P)
            .transpose(1, 2, 0, 3)
            .astype(ml_dtypes.bfloat16)
        )
        wk = (
            w_attn[:, C + s : C + s + HC]
            .reshape(CK, P, NP, P)
            .transpose(1, 2, 0, 3)
            .astype(ml_dtypes.bfloat16)
        )
        # [C, HC] -> [ki, ko, vcol]
        wv = (
            w_attn[:, 2 * C + s : 2 * C + s + HC]
            .reshape(CK, P, HC)
            .transpose(1, 0, 2)
            .astype(ml_dtypes.bfloat16)
        )
        # [HC, C] -> [ki, ko, co], bf16
        wp = (
            w_proj[s : s + HC, :]
            .reshape(HC // P, P, C)
            .transpose(1, 0, 2)
            .astype(ml_dtypes.bfloat16)
        )
        bq = b_attn[s : s + HC].reshape(NP, P).T
        bk = b_attn[C + s : C + s + HC].reshape(NP, P).T
        bv = np.broadcast_to(b_attn[2 * C + s : 2 * C + s + HC], (P, HC))
        bp = (
            np.broadcast_to(b_proj, (P, C))
            if g == 0
            else np.zeros((P, C), np.float32)
        )
        maps.append(
            {
                "xT": xT_b[b],
                "wq": np.ascontiguousarray(wq),
                "wk": np.ascontiguousarray(wk),
                "wv": np.ascontiguousarray(wv),
                "wp": np.ascontiguousarray(wp),
                "bq": np.ascontiguousarray(bq),
                "bk": np.ascontiguousarray(bk),
                "bv": np.ascontiguousarray(bv),
                "bp": np.ascontiguousarray(bp),
            }
        )
    return maps


def _run(inputs, trace=False, trace_cores=None):
    if "nc" not in _CACHE:
        _CACHE["nc"] = _build_nc()
    nc = _CACHE["nc"]
    res = run_bass_kernel_spmd(
        nc,
        _in_maps(inputs),
        list(range(N_CORES)),
        trace=trace,
        trace_cores=trace_cores,
    )
    # chunked RS ownership: per chunk, even core holds the first half of
    # the chunk's rows, odd core the second half
    out = np.empty((B, T, C), np.float32)
    for b in range(B):
        ev = res.results[2 * b]["out"].astype(np.float32)
        od = res.results[2 * b + 1]["out"].astype(np.float32)
        for t0r, tn in RS_LAYOUT:
            r0, rows = t0r * P, tn * P
            half = rows // 2
            out[b, r0 : r0 + half] = ev[r0 // 2 : r0 // 2 + half]
            out[b, r0 + half : r0 + rows] = od[r0 // 2 : r0 // 2 + half]
    return out, res


def kernel(**inputs):
    out, _ = _run(inputs)
    return out
